# revision 12
# baseline (speedup 1.0000x reference)
"""CharBiLSTM embedder on 8 Trainium2 NeuronCores (Bass/Tile).

Strategy (v3)
-------------
Data-parallel over words with an *equalized-length* assignment: words are
bucketed by length; each length class is spread round-robin over the 8
cores, padded with dummy words so every core has an identical length
profile. One shared SPMD program then fits all cores, and each word's
final LSTM state can be extracted on-device at its last valid step.

Per core: 4 blocks of W=512 words sorted ascending by length. At step t
only the suffix of A = W - a4 columns with len > t is computed. Both
directions are forward scans (the host reverses each word's chars for
the backward pass).

Engine placement per (block, step):
- The input-side gate contribution G[char] = emb[char] @ Wih^T + bias is
  gathered on the HOST (bias folded in; the g-chunk is pre-scaled by 2 so
  every gate activation is a plain Sigmoid) and DMA-streamed as a bf16
  [128, 8, A] tile. An identity matmul injects it into PSUM; 4 Whh
  matmuls per direction accumulate the recurrence on top. PSUM is split
  4+4 banks between the two directions so the PE can fill one
  direction's gates while the Scalar engine drains the other's.
- Scalar engine: one Sigmoid per direction over its 4 gate banks
  (tanh(g) = 2*sigmoid(2g) - 1 is fixed up on the DVE), one merged Tanh
  over both directions' cell states. 3 activation instructions/step.
- DVE: 4 merged (both-direction) ops for the cell update.
- GpSimd: h = o * tanh(c) and the final-state copy for words whose last
  valid char is this step (a contiguous column range, since words are
  sorted by length).

Outputs: per-block [128, 2dirs, W] final-state tiles DMA'd once per
block; host scatters them back to the original word order.
"""

import os
import sys

sys.path.insert(0, "/opt/trn_rl_repo")

import numpy as np
import ml_dtypes

import concourse.bacc as bacc
import concourse.tile as tile
import concourse.mybir as mybir
from concourse.bass_utils import run_bass_kernel_spmd

V, E, H = 256, 64, 128
N, L = 16384, 24
NCORES = 8
NPC = N // NCORES          # word slots per core
W = 512                    # words per block
NBLK = NPC // W
FP32 = mybir.dt.float32
BF16 = mybir.dt.bfloat16
FP16 = mybir.dt.float16
AF = mybir.ActivationFunctionType
OP = mybir.AluOpType
BF16NP = ml_dtypes.bfloat16


def _assign(lengths):
    """Equalized per-length word assignment.

    Returns (slots [NCORES, NPC] of word ids or -1, prof [NPC] shared
    ascending effective-length profile)."""
    bylen = [np.nonzero(lengths == l)[0] for l in range(L + 1)]
    n = [0] * (L + 1)
    for l in range(1, L + 1):
        n[l] = -(-len(bylen[l]) // NCORES)
    used = sum(n[1:])
    assert used <= NPC, (used, NPC)
    nz = NPC - used
    prof = np.concatenate(
        [np.zeros(nz, np.int32)]
        + [np.full(n[l], l, np.int32) for l in range(1, L + 1)])
    slots = -np.ones((NCORES, NPC), np.int64)
    pos = nz
    for l in range(1, L + 1):
        wl = bylen[l]
        for k in range(NCORES):
            wk = wl[k::NCORES]
            slots[k, pos:pos + len(wk)] = wk
        pos += n[l]
    # stripe the sorted profile across all 4 blocks so every block runs the
    # full 24 steps with ~1/4 of the words active: 4 dependency chains stay
    # alive at every t (A shrinks instead of blocks dying).
    perm = np.concatenate([np.arange(b, NPC, NBLK) for b in range(NBLK)])
    return slots[:, perm], prof[perm]


def _build_structure(prof):
    st = {"blocks": [], "TOT": 0}
    for b in range(NBLK):
        bl = prof[b * W:(b + 1) * W]
        lmax = int(bl[-1])
        steps = []
        off = 0
        for t in range(lmax):
            a4 = int(np.searchsorted(bl, t, side="right"))
            steps.append({"t": t, "a4": a4, "A": W - a4, "off": off})
            off += W - a4
        st["blocks"].append({"lmax": lmax, "steps": steps, "total": off,
                             "base": st["TOT"]})
        st["TOT"] += off
    return st


def _build_program(st):
    nc = bacc.Bacc("TRN2")
    TOT = max(st["TOT"], 1)
    LMAX = max((blk["lmax"] for blk in st["blocks"]), default=0)

    # weights: 8 WH chunks [128,128] (f: g,i,f,o then b: g,i,f,o) + identity
    w_d = nc.dram_tensor("wts", [128, 9 * 128], BF16, kind="ExternalInput")
    g_d = nc.dram_tensor("gin", [128, 8, TOT], BF16, kind="ExternalInput")
    out_d = nc.dram_tensor("out", [128, 2, NPC], BF16, kind="ExternalOutput")

    with tile.TileContext(nc) as tc:
        with (
            tc.tile_pool(name="const", bufs=1) as const_p,
            tc.tile_pool(name="g", bufs=4) as g_p,
            tc.tile_pool(name="state", bufs=1) as state_p,
            tc.tile_pool(name="scr", bufs=3) as scr_p,
            tc.tile_pool(name="ps", bufs=1, space="PSUM") as ps_p,
        ):
            w_sb = const_p.tile([128, 9 * 128], BF16)
            nc.sync.dma_start(w_sb[:], w_d[:])
            WH = [w_sb[:, c * 128:(c + 1) * 128] for c in range(8)]
            I128 = w_sb[:, 8 * 128:9 * 128]

            h_t, c_t = {}, {}
            for b in range(NBLK):
                if st["blocks"][b]["lmax"] == 0:
                    continue
                h_t[b] = state_p.tile([128, 2, W], BF16, name=f"h{b}")
                c_t[b] = state_p.tile([128, 2, W], FP16, name=f"c{b}")

            for t in range(LMAX):
                for b in range(NBLK):
                    blk = st["blocks"][b]
                    if t >= blk["lmax"]:
                        continue
                    s = blk["steps"][t]
                    a4, A = s["a4"], s["A"]
                    gbase = blk["base"] + s["off"]
                    h, c = h_t[b], c_t[b]

                    gt = g_p.tile([128, 8, W], BF16, tag="g",
                                  name=f"g{b}_{t}")
                    nc.sync.dma_start(gt[:, :, 0:A],
                                      g_d[:, :, gbase:gbase + A])
                    gsb = scr_p.tile([128, 8, W], FP16, tag="gsb",
                                     name=f"gsb{b}_{t}")
                    th = scr_p.tile([128, 2, W], FP16, tag="th",
                                    name=f"th{b}_{t}")
                    t1 = scr_p.tile([128, 2, W], FP16, tag="t1",
                                    name=f"t1{b}_{t}")
                    tig = scr_p.tile([128, 2, W], FP16, tag="tig",
                                     name=f"tig{b}_{t}")
                    cf2 = scr_p.tile([128, 2, W], FP16, tag="cf2",
                                     name=f"cf2{b}_{t}")
                    for d in (0, 1):
                        ps = ps_p.tile([128, 4, W], FP32, tag=f"ps{d}",
                                       name=f"ps{d}_{b}_{t}")
                        for j in range(4):
                            nc.tensor.matmul(ps[:, j, 0:A], I128,
                                             gt[:, 2 * j + d, 0:A],
                                             start=True, stop=(t == 0))
                        if t > 0:
                            for j in range(4):
                                nc.tensor.matmul(ps[:, j, 0:A], WH[4 * d + j],
                                                 h[:, d, a4:W],
                                                 start=False, stop=True)
                        # banks g,i,f,o of dir d -> gsb banks 2j+d
                        nc.scalar.activation(gsb[:, d:8:2, 0:A],
                                             ps[:, :, 0:A], AF.Sigmoid)
                    g_pair = gsb[:, 0:2, 0:A]
                    i_pair = gsb[:, 2:4, 0:A]
                    f_pair = gsb[:, 4:6, 0:A]
                    o_pair = gsb[:, 6:8, 0:A]
                    c_al = c[:, :, a4:W]
                    nc.vector.scalar_tensor_tensor(t1[:, :, 0:A], g_pair, 2.0,
                                                   i_pair, op0=OP.mult,
                                                   op1=OP.mult)
                    if t > 0:
                        # cf2 on gpsimd runs parallel to t1/tig on the DVE
                        nc.gpsimd.tensor_tensor(cf2[:, :, 0:A], c_al, f_pair,
                                                op=OP.mult)
                        nc.vector.tensor_tensor(tig[:, :, 0:A], t1[:, :, 0:A],
                                                i_pair, op=OP.subtract)
                        nc.vector.tensor_tensor(c_al, cf2[:, :, 0:A],
                                                tig[:, :, 0:A], op=OP.add)
                    else:
                        nc.vector.tensor_tensor(c_al, t1[:, :, 0:A], i_pair,
                                                op=OP.subtract)
                    nc.scalar.activation(th[:, :, 0:A], c_al, AF.Tanh)
                    nc.gpsimd.tensor_tensor(h[:, :, a4:W], o_pair,
                                            th[:, :, 0:A], op=OP.mult)
                    if t == blk["lmax"] - 1:
                        nc.sync.dma_start(out_d[:, :, b * W:(b + 1) * W],
                                          h[:])
    nc.compile()
    return nc


def kernel(char_indices, lengths, emb_table, Wih_f, Whh_f, bih_f, bhh_f,
           Wih_b, Whh_b, bih_b, bhh_b):
    char_indices = np.asarray(char_indices).astype(np.int32)
    lengths = np.asarray(lengths).astype(np.int32)

    slots, prof = _assign(lengths)
    st = _build_structure(prof)
    TOT = max(st["TOT"], 1)

    # --- per-core char arrays (fwd and per-word-reversed bwd) ---
    posL = np.arange(L)[None, :]
    valid = posL < prof[:, None]
    rev_idx = np.clip(prof[:, None] - 1 - posL, 0, L - 1)
    cf, cb = [], []
    for k in range(NCORES):
        sw = slots[k]
        chw = np.where(sw[:, None] >= 0, char_indices[np.maximum(sw, 0)], 0)
        f = np.where(valid, chw, 0)
        bwd = np.where(valid, np.take_along_axis(chw, rev_idx, axis=1), 0)
        cf.append(f)
        cb.append(bwd)

    # --- gate tables: G[c] = emb[c] @ Wih^T + bias, chunk order g,i,f,o ---
    emb = np.asarray(emb_table, np.float32)

    def mk(Wih, bih, bhh, Whh):
        Gt = emb @ np.asarray(Wih, np.float32).T \
            + (np.asarray(bih, np.float32) + np.asarray(bhh, np.float32))[None]
        rows = [slice(256, 384), slice(0, 128), slice(128, 256),
                slice(384, 512)]  # PyTorch i,f,g,o -> our g,i,f,o
        G4 = np.stack([Gt[:, r] for r in rows], axis=1)  # [256, 4, 128]
        G4[:, 0, :] *= 2.0  # tanh(g) = 2*sigmoid(2g) - 1
        wh = np.stack([np.ascontiguousarray(np.asarray(Whh, np.float32)[r].T)
                       for r in rows], axis=0)  # [4, 128k, 128m]
        wh[0] *= 2.0
        return G4.astype(BF16NP), wh

    G4f, whf = mk(Wih_f, bih_f, bhh_f, Whh_f)
    G4b, whb = mk(Wih_b, bih_b, bhh_b, Whh_b)

    w_all = np.concatenate(
        [whf[j] for j in range(4)] + [whb[j] for j in range(4)]
        + [np.eye(128, dtype=np.float32)], axis=1).astype(BF16NP)

    # --- per-core gathered G stream [128, 8, TOT] ---
    # flat (word-col, t) pairs in stream order; stream index == arange(TOT)
    wcol_all, t_all = [], []
    for b in range(NBLK):
        blk = st["blocks"][b]
        for s in blk["steps"]:
            wcol_all.append(np.arange(s["a4"], W) + b * W)
            t_all.append(np.full(s["A"], s["t"]))
    wcol_all = (np.concatenate(wcol_all) if wcol_all
                else np.zeros(0, np.int64))
    t_all = np.concatenate(t_all) if len(wcol_all) else np.zeros(0, np.int64)

    g_in = []
    for k in range(NCORES):
        gi = np.empty((128, 8, TOT), BF16NP)
        if len(wcol_all):
            chf = cf[k][wcol_all, t_all]
            chb = cb[k][wcol_all, t_all]
            # [TOT, 4, 128] -> [128, 4, TOT]
            gi[:, 0::2, :len(wcol_all)] = np.transpose(G4f[chf], (2, 1, 0))
            gi[:, 1::2, :len(wcol_all)] = np.transpose(G4b[chb], (2, 1, 0))
        g_in.append(gi)

    nc = _build_program(st)
    in_maps = [{"wts": w_all, "gin": g_in[k]} for k in range(NCORES)]
    trace = os.environ.get("LSTM_TRACE") == "1"
    res = run_bass_kernel_spmd(nc, in_maps, core_ids=list(range(NCORES)),
                               trace=trace)
    if trace and res.exec_time_ns is not None:
        print(f"HW exec time: {res.exec_time_ns} ns")
        print(f"HW exec time mean: {res.mean_exec_time_ns} ns")
        if res.instructions_and_trace:
            print(f"trace: {res.instructions_and_trace[1]}")

    out = np.zeros((N, 2 * H), np.float32)
    for k in range(NCORES):
        ob = np.asarray(res.results[k]["out"]).astype(np.float32)
        sw = slots[k]
        real = np.nonzero((sw >= 0) & (prof > 0))[0]
        wid = sw[real]
        out[wid, 0:H] = ob[:, 0, real].T
        out[wid, H:2 * H] = ob[:, 1, real].T
    return out


# revision 15
# speedup vs baseline: 1.1509x; 1.1509x over previous
"""CharBiLSTM embedder on 8 Trainium2 NeuronCores (Bass/Tile).

Strategy (v3)
-------------
Data-parallel over words with an *equalized-length* assignment: words are
bucketed by length; each length class is spread round-robin over the 8
cores, padded with dummy words so every core has an identical length
profile. One shared SPMD program then fits all cores, and each word's
final LSTM state can be extracted on-device at its last valid step.

Per core: 4 blocks of W=512 words sorted ascending by length. At step t
only the suffix of A = W - a4 columns with len > t is computed. Both
directions are forward scans (the host reverses each word's chars for
the backward pass).

Engine placement per (block, step):
- The input-side gate contribution G[char] = emb[char] @ Wih^T + bias is
  gathered on the HOST (bias folded in; the g-chunk is pre-scaled by 2 so
  every gate activation is a plain Sigmoid) and DMA-streamed as a bf16
  [128, 8, A] tile. An identity matmul injects it into PSUM; 4 Whh
  matmuls per direction accumulate the recurrence on top. PSUM is split
  4+4 banks between the two directions so the PE can fill one
  direction's gates while the Scalar engine drains the other's.
- Scalar engine: one Sigmoid per direction over its 4 gate banks
  (tanh(g) = 2*sigmoid(2g) - 1 is fixed up on the DVE), one merged Tanh
  over both directions' cell states. 3 activation instructions/step.
- DVE: 4 merged (both-direction) ops for the cell update.
- GpSimd: h = o * tanh(c) and the final-state copy for words whose last
  valid char is this step (a contiguous column range, since words are
  sorted by length).

Outputs: per-block [128, 2dirs, W] final-state tiles DMA'd once per
block; host scatters them back to the original word order.
"""

import os
import sys

sys.path.insert(0, "/opt/trn_rl_repo")

import numpy as np
import ml_dtypes

import concourse.bacc as bacc
import concourse.tile as tile
import concourse.mybir as mybir
from concourse.bass_utils import run_bass_kernel_spmd

V, E, H = 256, 64, 128
N, L = 16384, 24
NCORES = 8
NPC = N // NCORES          # word slots per core
W = 256                    # words per block
NBLK = NPC // W
FP32 = mybir.dt.float32
BF16 = mybir.dt.bfloat16
FP16 = mybir.dt.float16
AF = mybir.ActivationFunctionType
OP = mybir.AluOpType
BF16NP = ml_dtypes.bfloat16


def _assign(lengths):
    """Equalized per-length word assignment.

    Returns (slots [NCORES, NPC] of word ids or -1, prof [NPC] shared
    ascending effective-length profile)."""
    bylen = [np.nonzero(lengths == l)[0] for l in range(L + 1)]
    n = [0] * (L + 1)
    for l in range(1, L + 1):
        n[l] = -(-len(bylen[l]) // NCORES)
    used = sum(n[1:])
    assert used <= NPC, (used, NPC)
    nz = NPC - used
    prof = np.concatenate(
        [np.zeros(nz, np.int32)]
        + [np.full(n[l], l, np.int32) for l in range(1, L + 1)])
    slots = -np.ones((NCORES, NPC), np.int64)
    pos = nz
    for l in range(1, L + 1):
        wl = bylen[l]
        for k in range(NCORES):
            wk = wl[k::NCORES]
            slots[k, pos:pos + len(wk)] = wk
        pos += n[l]
    return slots, prof


def _build_structure(prof):
    st = {"blocks": [], "TOT": 0}
    for b in range(NBLK):
        bl = prof[b * W:(b + 1) * W]
        lmax = int(bl[-1])
        steps = []
        off = 0
        for t in range(lmax):
            a4 = int(np.searchsorted(bl, t, side="right"))
            steps.append({"t": t, "a4": a4, "A": W - a4, "off": off})
            off += W - a4
        st["blocks"].append({"lmax": lmax, "steps": steps, "total": off,
                             "base": st["TOT"]})
        st["TOT"] += off
    return st


def _build_program(st):
    nc = bacc.Bacc("TRN2")
    TOT = max(st["TOT"], 1)
    LMAX = max((blk["lmax"] for blk in st["blocks"]), default=0)

    # weights: 8 WH chunks [128,128] (f: g,i,f,o then b: g,i,f,o) + identity
    w_d = nc.dram_tensor("wts", [128, 9 * 128], BF16, kind="ExternalInput")
    g_d = nc.dram_tensor("gin", [128, 8, TOT], BF16, kind="ExternalInput")
    out_d = nc.dram_tensor("out", [128, 2, NPC], BF16, kind="ExternalOutput")

    with tile.TileContext(nc) as tc:
        with (
            tc.tile_pool(name="const", bufs=1) as const_p,
            tc.tile_pool(name="g", bufs=4) as g_p,
            tc.tile_pool(name="state", bufs=1) as state_p,
            tc.tile_pool(name="scr", bufs=3) as scr_p,
            tc.tile_pool(name="ps", bufs=1, space="PSUM") as ps_p,
        ):
            w_sb = const_p.tile([128, 9 * 128], BF16)
            nc.sync.dma_start(w_sb[:], w_d[:])
            WH = [w_sb[:, c * 128:(c + 1) * 128] for c in range(8)]
            I128 = w_sb[:, 8 * 128:9 * 128]

            h_t, c_t = {}, {}
            for b in range(NBLK):
                if st["blocks"][b]["lmax"] == 0:
                    continue
                h_t[b] = state_p.tile([128, 2, W], BF16, name=f"h{b}")
                c_t[b] = state_p.tile([128, 2, W], FP16, name=f"c{b}")

            sctr = 0
            for t in range(LMAX):
                for b in range(NBLK):
                    blk = st["blocks"][b]
                    if t >= blk["lmax"]:
                        continue
                    s = blk["steps"][t]
                    a4, A = s["a4"], s["A"]
                    gbase = blk["base"] + s["off"]
                    h, c = h_t[b], c_t[b]

                    gt = g_p.tile([128, 8, W], BF16, tag="g",
                                  name=f"g{b}_{t}")
                    nc.sync.dma_start(gt[:, :, 0:A],
                                      g_d[:, :, gbase:gbase + A])
                    gsb = scr_p.tile([128, 8, W], FP16, tag="gsb",
                                     name=f"gsb{b}_{t}")
                    th = scr_p.tile([128, 2, W], FP16, tag="th",
                                    name=f"th{b}_{t}")
                    u2 = scr_p.tile([128, 2, W], FP16, tag="u2",
                                    name=f"u2{b}_{t}")
                    cf2 = scr_p.tile([128, 2, W], FP16, tag="cf2",
                                     name=f"cf2{b}_{t}")
                    for d in (0, 1):
                        ps = ps_p.tile([128, 4, W], FP32,
                                       tag=f"ps{sctr % 4}",
                                       name=f"ps{b}_{t}_{d}")
                        sctr += 1
                        # chunks j=0,1 share a 2KB PSUM bank (j=2,3 the
                        # other): start=True zeroes the WHOLE bank, so only
                        # the first chunk per bank opens with start=True.
                        for j in range(4):
                            nc.tensor.matmul(ps[:, j, 0:A], I128,
                                             gt[:, 2 * j + d, 0:A],
                                             start=(j % 2 == 0),
                                             stop=(t == 0),
                                             skip_group_check=True)
                        if t > 0:
                            for j in range(4):
                                nc.tensor.matmul(ps[:, j, 0:A], WH[4 * d + j],
                                                 h[:, d, a4:W],
                                                 start=False, stop=True,
                                                 skip_group_check=True)
                        # banks g,i,f,o of dir d -> gsb banks 2j+d
                        nc.scalar.activation(gsb[:, d:8:2, 0:A],
                                             ps[:, :, 0:A], AF.Sigmoid)
                    g_pair = gsb[:, 0:2, 0:A]
                    i_pair = gsb[:, 2:4, 0:A]
                    f_pair = gsb[:, 4:6, 0:A]
                    o_pair = gsb[:, 6:8, 0:A]
                    c_al = c[:, :, a4:W]
                    # u = (sigmoid(2g) - 0.5) * i;  c = 2u + f*c'
                    nc.vector.scalar_tensor_tensor(u2[:, :, 0:A], g_pair, 0.5,
                                                   i_pair, op0=OP.subtract,
                                                   op1=OP.mult)
                    if t > 0:
                        # cf2 on gpsimd runs parallel to u on the DVE
                        nc.gpsimd.tensor_tensor(cf2[:, :, 0:A], c_al, f_pair,
                                                op=OP.mult)
                        nc.vector.scalar_tensor_tensor(c_al, u2[:, :, 0:A],
                                                       2.0, cf2[:, :, 0:A],
                                                       op0=OP.mult,
                                                       op1=OP.add)
                    else:
                        nc.vector.tensor_scalar_mul(c_al, u2[:, :, 0:A], 2.0)
                    nc.scalar.activation(th[:, :, 0:A], c_al, AF.Tanh)
                    nc.vector.tensor_tensor(h[:, :, a4:W], o_pair,
                                            th[:, :, 0:A], op=OP.mult)
                    if t == blk["lmax"] - 1:
                        nc.sync.dma_start(out_d[:, :, b * W:(b + 1) * W],
                                          h[:])
    nc.compile()
    return nc


def kernel(char_indices, lengths, emb_table, Wih_f, Whh_f, bih_f, bhh_f,
           Wih_b, Whh_b, bih_b, bhh_b):
    char_indices = np.asarray(char_indices).astype(np.int32)
    lengths = np.asarray(lengths).astype(np.int32)

    slots, prof = _assign(lengths)
    st = _build_structure(prof)
    TOT = max(st["TOT"], 1)

    # --- per-core char arrays (fwd and per-word-reversed bwd) ---
    posL = np.arange(L)[None, :]
    valid = posL < prof[:, None]
    rev_idx = np.clip(prof[:, None] - 1 - posL, 0, L - 1)
    cf, cb = [], []
    for k in range(NCORES):
        sw = slots[k]
        chw = np.where(sw[:, None] >= 0, char_indices[np.maximum(sw, 0)], 0)
        f = np.where(valid, chw, 0)
        bwd = np.where(valid, np.take_along_axis(chw, rev_idx, axis=1), 0)
        cf.append(f)
        cb.append(bwd)

    # --- gate tables: G[c] = emb[c] @ Wih^T + bias, chunk order g,i,f,o ---
    emb = np.asarray(emb_table, np.float32)

    def mk(Wih, bih, bhh, Whh):
        Gt = emb @ np.asarray(Wih, np.float32).T \
            + (np.asarray(bih, np.float32) + np.asarray(bhh, np.float32))[None]
        rows = [slice(256, 384), slice(0, 128), slice(128, 256),
                slice(384, 512)]  # PyTorch i,f,g,o -> our g,i,f,o
        G4 = np.stack([Gt[:, r] for r in rows], axis=1)  # [256, 4, 128]
        G4[:, 0, :] *= 2.0  # tanh(g) = 2*sigmoid(2g) - 1
        wh = np.stack([np.ascontiguousarray(np.asarray(Whh, np.float32)[r].T)
                       for r in rows], axis=0)  # [4, 128k, 128m]
        wh[0] *= 2.0
        return G4.astype(BF16NP), wh

    G4f, whf = mk(Wih_f, bih_f, bhh_f, Whh_f)
    G4b, whb = mk(Wih_b, bih_b, bhh_b, Whh_b)

    w_all = np.concatenate(
        [whf[j] for j in range(4)] + [whb[j] for j in range(4)]
        + [np.eye(128, dtype=np.float32)], axis=1).astype(BF16NP)

    # --- per-core gathered G stream [128, 8, TOT] ---
    # flat (word-col, t) pairs in stream order; stream index == arange(TOT)
    wcol_all, t_all = [], []
    for b in range(NBLK):
        blk = st["blocks"][b]
        for s in blk["steps"]:
            wcol_all.append(np.arange(s["a4"], W) + b * W)
            t_all.append(np.full(s["A"], s["t"]))
    wcol_all = (np.concatenate(wcol_all) if wcol_all
                else np.zeros(0, np.int64))
    t_all = np.concatenate(t_all) if len(wcol_all) else np.zeros(0, np.int64)

    g_in = []
    for k in range(NCORES):
        gi = np.empty((128, 8, TOT), BF16NP)
        if len(wcol_all):
            chf = cf[k][wcol_all, t_all]
            chb = cb[k][wcol_all, t_all]
            # [TOT, 4, 128] -> [128, 4, TOT]
            gi[:, 0::2, :len(wcol_all)] = np.transpose(G4f[chf], (2, 1, 0))
            gi[:, 1::2, :len(wcol_all)] = np.transpose(G4b[chb], (2, 1, 0))
        g_in.append(gi)

    nc = _build_program(st)
    in_maps = [{"wts": w_all, "gin": g_in[k]} for k in range(NCORES)]
    trace = os.environ.get("LSTM_TRACE") == "1"
    res = run_bass_kernel_spmd(nc, in_maps, core_ids=list(range(NCORES)),
                               trace=trace)
    if trace and res.exec_time_ns is not None:
        print(f"HW exec time: {res.exec_time_ns} ns")
        print(f"HW exec time mean: {res.mean_exec_time_ns} ns")
        if res.instructions_and_trace:
            print(f"trace: {res.instructions_and_trace[1]}")

    out = np.zeros((N, 2 * H), np.float32)
    for k in range(NCORES):
        ob = np.asarray(res.results[k]["out"]).astype(np.float32)
        sw = slots[k]
        real = np.nonzero((sw >= 0) & (prof > 0))[0]
        wid = sw[real]
        out[wid, 0:H] = ob[:, 0, real].T
        out[wid, H:2 * H] = ob[:, 1, real].T
    return out


# revision 16
# speedup vs baseline: 1.1578x; 1.0060x over previous
"""CharBiLSTM embedder on 8 Trainium2 NeuronCores (Bass/Tile).

Strategy (v3)
-------------
Data-parallel over words with an *equalized-length* assignment: words are
bucketed by length; each length class is spread round-robin over the 8
cores, padded with dummy words so every core has an identical length
profile. One shared SPMD program then fits all cores, and each word's
final LSTM state can be extracted on-device at its last valid step.

Per core: 4 blocks of W=512 words sorted ascending by length. At step t
only the suffix of A = W - a4 columns with len > t is computed. Both
directions are forward scans (the host reverses each word's chars for
the backward pass).

Engine placement per (block, step):
- The input-side gate contribution G[char] = emb[char] @ Wih^T + bias is
  gathered on the HOST (bias folded in; the g-chunk is pre-scaled by 2 so
  every gate activation is a plain Sigmoid) and DMA-streamed as a bf16
  [128, 8, A] tile. An identity matmul injects it into PSUM; 4 Whh
  matmuls per direction accumulate the recurrence on top. PSUM is split
  4+4 banks between the two directions so the PE can fill one
  direction's gates while the Scalar engine drains the other's.
- Scalar engine: one Sigmoid per direction over its 4 gate banks
  (tanh(g) = 2*sigmoid(2g) - 1 is fixed up on the DVE), one merged Tanh
  over both directions' cell states. 3 activation instructions/step.
- DVE: 4 merged (both-direction) ops for the cell update.
- GpSimd: h = o * tanh(c) and the final-state copy for words whose last
  valid char is this step (a contiguous column range, since words are
  sorted by length).

Outputs: per-block [128, 2dirs, W] final-state tiles DMA'd once per
block; host scatters them back to the original word order.
"""

import os
import sys

sys.path.insert(0, "/opt/trn_rl_repo")

import numpy as np
import ml_dtypes

import concourse.bacc as bacc
import concourse.tile as tile
import concourse.mybir as mybir
from concourse.bass_utils import run_bass_kernel_spmd

V, E, H = 256, 64, 128
N, L = 16384, 24
NCORES = 8
NPC = N // NCORES          # word slots per core
W = 256                    # words per block
NBLK = NPC // W
FP32 = mybir.dt.float32
BF16 = mybir.dt.bfloat16
FP16 = mybir.dt.float16
AF = mybir.ActivationFunctionType
OP = mybir.AluOpType
BF16NP = ml_dtypes.bfloat16


def _assign(lengths):
    """Equalized per-length word assignment.

    Returns (slots [NCORES, NPC] of word ids or -1, prof [NPC] shared
    ascending effective-length profile)."""
    bylen = [np.nonzero(lengths == l)[0] for l in range(L + 1)]
    n = [0] * (L + 1)
    for l in range(1, L + 1):
        n[l] = -(-len(bylen[l]) // NCORES)
    used = sum(n[1:])
    assert used <= NPC, (used, NPC)
    nz = NPC - used
    prof = np.concatenate(
        [np.zeros(nz, np.int32)]
        + [np.full(n[l], l, np.int32) for l in range(1, L + 1)])
    slots = -np.ones((NCORES, NPC), np.int64)
    pos = nz
    for l in range(1, L + 1):
        wl = bylen[l]
        for k in range(NCORES):
            wk = wl[k::NCORES]
            slots[k, pos:pos + len(wk)] = wk
        pos += n[l]
    return slots, prof


def _build_structure(prof):
    st = {"blocks": [], "TOT": 0}
    for b in range(NBLK):
        bl = prof[b * W:(b + 1) * W]
        lmax = int(bl[-1])
        steps = []
        off = 0
        for t in range(lmax):
            a4 = int(np.searchsorted(bl, t, side="right"))
            steps.append({"t": t, "a4": a4, "A": W - a4, "off": off})
            off += W - a4
        st["blocks"].append({"lmax": lmax, "steps": steps, "total": off,
                             "base": st["TOT"]})
        st["TOT"] += off
    return st


def _build_program(st):
    nc = bacc.Bacc("TRN2")
    TOT = max(st["TOT"], 1)
    LMAX = max((blk["lmax"] for blk in st["blocks"]), default=0)

    # weights: 8 WH chunks [128,128] (f: g,i,f,o then b: g,i,f,o) + identity
    w_d = nc.dram_tensor("wts", [128, 9 * 128], BF16, kind="ExternalInput")
    g_d = nc.dram_tensor("gin", [128, 8, TOT], BF16, kind="ExternalInput")
    out_d = nc.dram_tensor("out", [128, 2, NPC], BF16, kind="ExternalOutput")

    with tile.TileContext(nc) as tc:
        with (
            tc.tile_pool(name="const", bufs=1) as const_p,
            tc.tile_pool(name="g", bufs=4) as g_p,
            tc.tile_pool(name="state", bufs=1) as state_p,
            tc.tile_pool(name="scr", bufs=3) as scr_p,
            tc.tile_pool(name="ps", bufs=1, space="PSUM") as ps_p,
        ):
            w_sb = const_p.tile([128, 9 * 128], BF16)
            nc.sync.dma_start(w_sb[:], w_d[:])
            WH = [w_sb[:, c * 128:(c + 1) * 128] for c in range(8)]
            I128 = w_sb[:, 8 * 128:9 * 128]

            h_t, c_t = {}, {}
            for b in range(NBLK):
                if st["blocks"][b]["lmax"] == 0:
                    continue
                h_t[b] = state_p.tile([128, 2, W], BF16, name=f"h{b}")
                c_t[b] = state_p.tile([128, 2, W], FP16, name=f"c{b}")

            sctr = 0
            for t in range(LMAX):
                for b in range(NBLK):
                    blk = st["blocks"][b]
                    if t >= blk["lmax"]:
                        continue
                    s = blk["steps"][t]
                    a4, A = s["a4"], s["A"]
                    gbase = blk["base"] + s["off"]
                    h, c = h_t[b], c_t[b]

                    gt = g_p.tile([128, 8, W], BF16, tag="g",
                                  name=f"g{b}_{t}")
                    nc.sync.dma_start(gt[:, :, 0:A],
                                      g_d[:, :, gbase:gbase + A])
                    gsb = scr_p.tile([128, 8, W], FP16, tag="gsb",
                                     name=f"gsb{b}_{t}")
                    th = scr_p.tile([128, 2, W], FP16, tag="th",
                                    name=f"th{b}_{t}")
                    u2 = scr_p.tile([128, 2, W], FP16, tag="u2",
                                    name=f"u2{b}_{t}")
                    cf2 = scr_p.tile([128, 2, W], FP16, tag="cf2",
                                     name=f"cf2{b}_{t}")
                    # one [128, 8, W] station (4 banks) per step, 2 rotating;
                    # bank 2j+d = (gate j, dir d), matching the gsb layout.
                    # Chunk pairs (d=0, d=1) share a 2KB PSUM bank and
                    # start=True zeroes the WHOLE bank, so only d=0 opens it.
                    ps = ps_p.tile([128, 8, W], FP32, tag=f"ps{sctr % 2}",
                                   name=f"ps{b}_{t}")
                    sctr += 1
                    for j in range(4):
                        for d in (0, 1):
                            nc.tensor.matmul(ps[:, 2 * j + d, 0:A], I128,
                                             gt[:, 2 * j + d, 0:A],
                                             start=(d == 0), stop=(t == 0),
                                             skip_group_check=True)
                    if t > 0:
                        for d in (0, 1):
                            for j in range(4):
                                nc.tensor.matmul(ps[:, 2 * j + d, 0:A],
                                                 WH[4 * d + j],
                                                 h[:, d, a4:W],
                                                 start=False, stop=True,
                                                 skip_group_check=True)
                    nc.scalar.activation(gsb[:, :, 0:A], ps[:, :, 0:A],
                                         AF.Sigmoid)
                    g_pair = gsb[:, 0:2, 0:A]
                    i_pair = gsb[:, 2:4, 0:A]
                    f_pair = gsb[:, 4:6, 0:A]
                    o_pair = gsb[:, 6:8, 0:A]
                    c_al = c[:, :, a4:W]
                    # u = (sigmoid(2g) - 0.5) * i;  c = 2u + f*c'
                    nc.vector.scalar_tensor_tensor(u2[:, :, 0:A], g_pair, 0.5,
                                                   i_pair, op0=OP.subtract,
                                                   op1=OP.mult)
                    if t > 0:
                        # cf2 on gpsimd runs parallel to u on the DVE
                        nc.gpsimd.tensor_tensor(cf2[:, :, 0:A], c_al, f_pair,
                                                op=OP.mult)
                        nc.vector.scalar_tensor_tensor(c_al, u2[:, :, 0:A],
                                                       2.0, cf2[:, :, 0:A],
                                                       op0=OP.mult,
                                                       op1=OP.add)
                    else:
                        nc.vector.tensor_scalar_mul(c_al, u2[:, :, 0:A], 2.0)
                    nc.scalar.activation(th[:, :, 0:A], c_al, AF.Tanh)
                    nc.vector.tensor_tensor(h[:, :, a4:W], o_pair,
                                            th[:, :, 0:A], op=OP.mult)
                    if t == blk["lmax"] - 1:
                        nc.sync.dma_start(out_d[:, :, b * W:(b + 1) * W],
                                          h[:])
    nc.compile()
    return nc


def kernel(char_indices, lengths, emb_table, Wih_f, Whh_f, bih_f, bhh_f,
           Wih_b, Whh_b, bih_b, bhh_b):
    char_indices = np.asarray(char_indices).astype(np.int32)
    lengths = np.asarray(lengths).astype(np.int32)

    slots, prof = _assign(lengths)
    st = _build_structure(prof)
    TOT = max(st["TOT"], 1)

    # --- per-core char arrays (fwd and per-word-reversed bwd) ---
    posL = np.arange(L)[None, :]
    valid = posL < prof[:, None]
    rev_idx = np.clip(prof[:, None] - 1 - posL, 0, L - 1)
    cf, cb = [], []
    for k in range(NCORES):
        sw = slots[k]
        chw = np.where(sw[:, None] >= 0, char_indices[np.maximum(sw, 0)], 0)
        f = np.where(valid, chw, 0)
        bwd = np.where(valid, np.take_along_axis(chw, rev_idx, axis=1), 0)
        cf.append(f)
        cb.append(bwd)

    # --- gate tables: G[c] = emb[c] @ Wih^T + bias, chunk order g,i,f,o ---
    emb = np.asarray(emb_table, np.float32)

    def mk(Wih, bih, bhh, Whh):
        Gt = emb @ np.asarray(Wih, np.float32).T \
            + (np.asarray(bih, np.float32) + np.asarray(bhh, np.float32))[None]
        rows = [slice(256, 384), slice(0, 128), slice(128, 256),
                slice(384, 512)]  # PyTorch i,f,g,o -> our g,i,f,o
        G4 = np.stack([Gt[:, r] for r in rows], axis=1)  # [256, 4, 128]
        G4[:, 0, :] *= 2.0  # tanh(g) = 2*sigmoid(2g) - 1
        wh = np.stack([np.ascontiguousarray(np.asarray(Whh, np.float32)[r].T)
                       for r in rows], axis=0)  # [4, 128k, 128m]
        wh[0] *= 2.0
        return G4.astype(BF16NP), wh

    G4f, whf = mk(Wih_f, bih_f, bhh_f, Whh_f)
    G4b, whb = mk(Wih_b, bih_b, bhh_b, Whh_b)

    w_all = np.concatenate(
        [whf[j] for j in range(4)] + [whb[j] for j in range(4)]
        + [np.eye(128, dtype=np.float32)], axis=1).astype(BF16NP)

    # --- per-core gathered G stream [128, 8, TOT] ---
    # flat (word-col, t) pairs in stream order; stream index == arange(TOT)
    wcol_all, t_all = [], []
    for b in range(NBLK):
        blk = st["blocks"][b]
        for s in blk["steps"]:
            wcol_all.append(np.arange(s["a4"], W) + b * W)
            t_all.append(np.full(s["A"], s["t"]))
    wcol_all = (np.concatenate(wcol_all) if wcol_all
                else np.zeros(0, np.int64))
    t_all = np.concatenate(t_all) if len(wcol_all) else np.zeros(0, np.int64)

    g_in = []
    for k in range(NCORES):
        gi = np.empty((128, 8, TOT), BF16NP)
        if len(wcol_all):
            chf = cf[k][wcol_all, t_all]
            chb = cb[k][wcol_all, t_all]
            # [TOT, 4, 128] -> [128, 4, TOT]
            gi[:, 0::2, :len(wcol_all)] = np.transpose(G4f[chf], (2, 1, 0))
            gi[:, 1::2, :len(wcol_all)] = np.transpose(G4b[chb], (2, 1, 0))
        g_in.append(gi)

    nc = _build_program(st)
    in_maps = [{"wts": w_all, "gin": g_in[k]} for k in range(NCORES)]
    trace = os.environ.get("LSTM_TRACE") == "1"
    res = run_bass_kernel_spmd(nc, in_maps, core_ids=list(range(NCORES)),
                               trace=trace)
    if trace and res.exec_time_ns is not None:
        print(f"HW exec time: {res.exec_time_ns} ns")
        print(f"HW exec time mean: {res.mean_exec_time_ns} ns")
        if res.instructions_and_trace:
            print(f"trace: {res.instructions_and_trace[1]}")

    out = np.zeros((N, 2 * H), np.float32)
    for k in range(NCORES):
        ob = np.asarray(res.results[k]["out"]).astype(np.float32)
        sw = slots[k]
        real = np.nonzero((sw >= 0) & (prof > 0))[0]
        wid = sw[real]
        out[wid, 0:H] = ob[:, 0, real].T
        out[wid, H:2 * H] = ob[:, 1, real].T
    return out


# revision 18
# speedup vs baseline: 1.2052x; 1.0409x over previous
"""CharBiLSTM embedder on 8 Trainium2 NeuronCores (Bass/Tile).

Strategy (v3)
-------------
Data-parallel over words with an *equalized-length* assignment: words are
bucketed by length; each length class is spread round-robin over the 8
cores, padded with dummy words so every core has an identical length
profile. One shared SPMD program then fits all cores, and each word's
final LSTM state can be extracted on-device at its last valid step.

Per core: 4 blocks of W=512 words sorted ascending by length. At step t
only the suffix of A = W - a4 columns with len > t is computed. Both
directions are forward scans (the host reverses each word's chars for
the backward pass).

Engine placement per (block, step):
- The input-side gate contribution G[char] = emb[char] @ Wih^T + bias is
  gathered on the HOST (bias folded in; the g-chunk is pre-scaled by 2 so
  every gate activation is a plain Sigmoid) and DMA-streamed as a bf16
  [128, 8, A] tile. An identity matmul injects it into PSUM; 4 Whh
  matmuls per direction accumulate the recurrence on top. PSUM is split
  4+4 banks between the two directions so the PE can fill one
  direction's gates while the Scalar engine drains the other's.
- Scalar engine: one Sigmoid per direction over its 4 gate banks
  (tanh(g) = 2*sigmoid(2g) - 1 is fixed up on the DVE), one merged Tanh
  over both directions' cell states. 3 activation instructions/step.
- DVE: 4 merged (both-direction) ops for the cell update.
- GpSimd: h = o * tanh(c) and the final-state copy for words whose last
  valid char is this step (a contiguous column range, since words are
  sorted by length).

Outputs: per-block [128, 2dirs, W] final-state tiles DMA'd once per
block; host scatters them back to the original word order.
"""

import os
import sys

sys.path.insert(0, "/opt/trn_rl_repo")

import numpy as np
import ml_dtypes

import concourse.bacc as bacc
import concourse.tile as tile
import concourse.mybir as mybir
from concourse.bass_utils import run_bass_kernel_spmd

V, E, H = 256, 64, 128
N, L = 16384, 24
NCORES = 8
NPC = N // NCORES          # word slots per core
W = 256                    # words per block
NBLK = NPC // W
FP32 = mybir.dt.float32
BF16 = mybir.dt.bfloat16
FP16 = mybir.dt.float16
AF = mybir.ActivationFunctionType
OP = mybir.AluOpType
BF16NP = ml_dtypes.bfloat16


def _assign(lengths):
    """Equalized per-length word assignment.

    Returns (slots [NCORES, NPC] of word ids or -1, prof [NPC] shared
    ascending effective-length profile)."""
    bylen = [np.nonzero(lengths == l)[0] for l in range(L + 1)]
    n = [0] * (L + 1)
    for l in range(1, L + 1):
        n[l] = -(-len(bylen[l]) // NCORES)
    used = sum(n[1:])
    assert used <= NPC, (used, NPC)
    nz = NPC - used
    prof = np.concatenate(
        [np.zeros(nz, np.int32)]
        + [np.full(n[l], l, np.int32) for l in range(1, L + 1)])
    slots = -np.ones((NCORES, NPC), np.int64)
    pos = nz
    for l in range(1, L + 1):
        wl = bylen[l]
        for k in range(NCORES):
            wk = wl[k::NCORES]
            slots[k, pos:pos + len(wk)] = wk
        pos += n[l]
    # stripe the top two blocks (longest words) so both run all 24 steps
    # with half the columns: two critical chains with short instructions.
    top = NPC - 2 * W
    perm = np.concatenate([
        np.arange(top), np.arange(top, NPC, 2), np.arange(top + 1, NPC, 2)])
    return slots[:, perm], prof[perm]


def _build_structure(prof):
    st = {"blocks": [], "TOT": 0}
    for b in range(NBLK):
        bl = prof[b * W:(b + 1) * W]
        lmax = int(bl[-1])
        steps = []
        off = 0
        for t in range(lmax):
            a4 = int(np.searchsorted(bl, t, side="right"))
            steps.append({"t": t, "a4": a4, "A": W - a4, "off": off})
            off += W - a4
        st["blocks"].append({"lmax": lmax, "steps": steps, "total": off,
                             "base": st["TOT"]})
        st["TOT"] += off
    return st


def _build_program(st):
    nc = bacc.Bacc("TRN2")
    TOT = max(st["TOT"], 1)
    LMAX = max((blk["lmax"] for blk in st["blocks"]), default=0)

    # weights: 8 WH chunks [128,128] (f: g,i,f,o then b: g,i,f,o) + identity
    w_d = nc.dram_tensor("wts", [128, 9 * 128], BF16, kind="ExternalInput")
    g_d = nc.dram_tensor("gin", [128, 8, TOT], BF16, kind="ExternalInput")
    out_d = nc.dram_tensor("out", [128, 2, NPC], BF16, kind="ExternalOutput")

    with tile.TileContext(nc) as tc:
        with (
            tc.tile_pool(name="const", bufs=1) as const_p,
            tc.tile_pool(name="g", bufs=4) as g_p,
            tc.tile_pool(name="state", bufs=1) as state_p,
            tc.tile_pool(name="scr", bufs=3) as scr_p,
            tc.tile_pool(name="ps", bufs=1, space="PSUM") as ps_p,
        ):
            w_sb = const_p.tile([128, 9 * 128], BF16)
            nc.sync.dma_start(w_sb[:], w_d[:])
            WH = [w_sb[:, c * 128:(c + 1) * 128] for c in range(8)]
            I128 = w_sb[:, 8 * 128:9 * 128]

            h_t, c_t = {}, {}
            for b in range(NBLK):
                if st["blocks"][b]["lmax"] == 0:
                    continue
                h_t[b] = state_p.tile([128, 2, W], BF16, name=f"h{b}")
                c_t[b] = state_p.tile([128, 2, W], FP16, name=f"c{b}")

            sctr = 0
            for t in range(LMAX):
                # longest blocks first so the critical chain never queues
                # behind slack work on any engine
                for b in reversed(range(NBLK)):
                    blk = st["blocks"][b]
                    if t >= blk["lmax"]:
                        continue
                    s = blk["steps"][t]
                    a4, A = s["a4"], s["A"]
                    gbase = blk["base"] + s["off"]
                    h, c = h_t[b], c_t[b]

                    gt = g_p.tile([128, 8, W], BF16, tag="g",
                                  name=f"g{b}_{t}")
                    nc.sync.dma_start(gt[:, :, 0:A],
                                      g_d[:, :, gbase:gbase + A])
                    gsb = scr_p.tile([128, 8, W], FP16, tag="gsb",
                                     name=f"gsb{b}_{t}")
                    th = scr_p.tile([128, 2, W], FP16, tag="th",
                                    name=f"th{b}_{t}")
                    u2 = scr_p.tile([128, 2, W], FP16, tag="u2",
                                    name=f"u2{b}_{t}")
                    cf2 = scr_p.tile([128, 2, W], FP16, tag="cf2",
                                     name=f"cf2{b}_{t}")
                    # one [128, 8, W] station (4 banks) per step, 2 rotating;
                    # bank 2j+d = (gate j, dir d), matching the gsb layout.
                    # Chunk pairs (d=0, d=1) share a 2KB PSUM bank and
                    # start=True zeroes the WHOLE bank, so only d=0 opens it.
                    ps = ps_p.tile([128, 8, W], FP32, tag=f"ps{sctr % 2}",
                                   name=f"ps{b}_{t}")
                    sctr += 1
                    for j in range(4):
                        for d in (0, 1):
                            nc.tensor.matmul(ps[:, 2 * j + d, 0:A], I128,
                                             gt[:, 2 * j + d, 0:A],
                                             start=(d == 0), stop=(t == 0),
                                             skip_group_check=True)
                    if t > 0:
                        for d in (0, 1):
                            for j in range(4):
                                nc.tensor.matmul(ps[:, 2 * j + d, 0:A],
                                                 WH[4 * d + j],
                                                 h[:, d, a4:W],
                                                 start=False, stop=True,
                                                 skip_group_check=True)
                    nc.scalar.activation(gsb[:, :, 0:A], ps[:, :, 0:A],
                                         AF.Sigmoid)
                    g_pair = gsb[:, 0:2, 0:A]
                    i_pair = gsb[:, 2:4, 0:A]
                    f_pair = gsb[:, 4:6, 0:A]
                    o_pair = gsb[:, 6:8, 0:A]
                    c_al = c[:, :, a4:W]
                    # u = (sigmoid(2g) - 0.5) * i;  c = 2u + f*c'
                    nc.vector.scalar_tensor_tensor(u2[:, :, 0:A], g_pair, 0.5,
                                                   i_pair, op0=OP.subtract,
                                                   op1=OP.mult)
                    if t > 0:
                        # cf2 on gpsimd runs parallel to u on the DVE
                        nc.gpsimd.tensor_tensor(cf2[:, :, 0:A], c_al, f_pair,
                                                op=OP.mult)
                        nc.vector.scalar_tensor_tensor(c_al, u2[:, :, 0:A],
                                                       2.0, cf2[:, :, 0:A],
                                                       op0=OP.mult,
                                                       op1=OP.add)
                    else:
                        nc.vector.tensor_scalar_mul(c_al, u2[:, :, 0:A], 2.0)
                    nc.scalar.activation(th[:, :, 0:A], c_al, AF.Tanh)
                    nc.vector.tensor_tensor(h[:, :, a4:W], o_pair,
                                            th[:, :, 0:A], op=OP.mult)
                    if t == blk["lmax"] - 1:
                        nc.sync.dma_start(out_d[:, :, b * W:(b + 1) * W],
                                          h[:])
    nc.compile()
    return nc


def kernel(char_indices, lengths, emb_table, Wih_f, Whh_f, bih_f, bhh_f,
           Wih_b, Whh_b, bih_b, bhh_b):
    char_indices = np.asarray(char_indices).astype(np.int32)
    lengths = np.asarray(lengths).astype(np.int32)

    slots, prof = _assign(lengths)
    st = _build_structure(prof)
    TOT = max(st["TOT"], 1)

    # --- per-core char arrays (fwd and per-word-reversed bwd) ---
    posL = np.arange(L)[None, :]
    valid = posL < prof[:, None]
    rev_idx = np.clip(prof[:, None] - 1 - posL, 0, L - 1)
    cf, cb = [], []
    for k in range(NCORES):
        sw = slots[k]
        chw = np.where(sw[:, None] >= 0, char_indices[np.maximum(sw, 0)], 0)
        f = np.where(valid, chw, 0)
        bwd = np.where(valid, np.take_along_axis(chw, rev_idx, axis=1), 0)
        cf.append(f)
        cb.append(bwd)

    # --- gate tables: G[c] = emb[c] @ Wih^T + bias, chunk order g,i,f,o ---
    emb = np.asarray(emb_table, np.float32)

    def mk(Wih, bih, bhh, Whh):
        Gt = emb @ np.asarray(Wih, np.float32).T \
            + (np.asarray(bih, np.float32) + np.asarray(bhh, np.float32))[None]
        rows = [slice(256, 384), slice(0, 128), slice(128, 256),
                slice(384, 512)]  # PyTorch i,f,g,o -> our g,i,f,o
        G4 = np.stack([Gt[:, r] for r in rows], axis=1)  # [256, 4, 128]
        G4[:, 0, :] *= 2.0  # tanh(g) = 2*sigmoid(2g) - 1
        wh = np.stack([np.ascontiguousarray(np.asarray(Whh, np.float32)[r].T)
                       for r in rows], axis=0)  # [4, 128k, 128m]
        wh[0] *= 2.0
        return G4.astype(BF16NP), wh

    G4f, whf = mk(Wih_f, bih_f, bhh_f, Whh_f)
    G4b, whb = mk(Wih_b, bih_b, bhh_b, Whh_b)

    w_all = np.concatenate(
        [whf[j] for j in range(4)] + [whb[j] for j in range(4)]
        + [np.eye(128, dtype=np.float32)], axis=1).astype(BF16NP)

    # --- per-core gathered G stream [128, 8, TOT] ---
    # flat (word-col, t) pairs in stream order; stream index == arange(TOT)
    wcol_all, t_all = [], []
    for b in range(NBLK):
        blk = st["blocks"][b]
        for s in blk["steps"]:
            wcol_all.append(np.arange(s["a4"], W) + b * W)
            t_all.append(np.full(s["A"], s["t"]))
    wcol_all = (np.concatenate(wcol_all) if wcol_all
                else np.zeros(0, np.int64))
    t_all = np.concatenate(t_all) if len(wcol_all) else np.zeros(0, np.int64)

    g_in = []
    for k in range(NCORES):
        gi = np.empty((128, 8, TOT), BF16NP)
        if len(wcol_all):
            chf = cf[k][wcol_all, t_all]
            chb = cb[k][wcol_all, t_all]
            # [TOT, 4, 128] -> [128, 4, TOT]
            gi[:, 0::2, :len(wcol_all)] = np.transpose(G4f[chf], (2, 1, 0))
            gi[:, 1::2, :len(wcol_all)] = np.transpose(G4b[chb], (2, 1, 0))
        g_in.append(gi)

    nc = _build_program(st)
    in_maps = [{"wts": w_all, "gin": g_in[k]} for k in range(NCORES)]
    trace = os.environ.get("LSTM_TRACE") == "1"
    res = run_bass_kernel_spmd(nc, in_maps, core_ids=list(range(NCORES)),
                               trace=trace)
    if trace and res.exec_time_ns is not None:
        print(f"HW exec time: {res.exec_time_ns} ns")
        print(f"HW exec time mean: {res.mean_exec_time_ns} ns")
        if res.instructions_and_trace:
            print(f"trace: {res.instructions_and_trace[1]}")

    out = np.zeros((N, 2 * H), np.float32)
    for k in range(NCORES):
        ob = np.asarray(res.results[k]["out"]).astype(np.float32)
        sw = slots[k]
        real = np.nonzero((sw >= 0) & (prof > 0))[0]
        wid = sw[real]
        out[wid, 0:H] = ob[:, 0, real].T
        out[wid, H:2 * H] = ob[:, 1, real].T
    return out
